# revision 10
# baseline (speedup 1.0000x reference)
"""AdaptiveFFNMoE — expert-parallel Bass kernel on 8 TRN2 NeuronCores.

Split of work:
  host (numpy): LayerNorm, router softmax, adaptive top-k, top-2 select,
    per-expert token gather, final weighted scatter-add + residual.
  device (Bass/Tile, SPMD over 8 cores): the expert FFNs in bf16 with
    f32 PSUM accumulation, balanced across cores by splitting each
    expert's hidden dimension across a pair of cores.

Sharding (the FLOP-balancing is exact by construction):
  work unit = (expert, 512 hidden units). 120 units total, 15 per core.
  Expert pairs share two cores — unit counts 18+12, 16+14, 22+8, 20+10
  sum to 30 for every pair:
    cores 0,1: e5 (9 units each) + e2 (6 each)
    cores 2,3: e4 (8) + e3 (7)
    cores 4,5: e0 (4) + e7 (11)
    cores 6,7: e6 (10) + e1 (5)
  Token-slot widths are static ([1536]*9 + [1408]*6, max real load 1521);
  units find their token block / y block via register offsets loaded
  from a tiny per-core job table. Each core accumulates its partial y
  in an SBUF bf16 accumulator; the host sums the two partial y's of
  each expert pair, applies gate weights + bo, and scatters.

Device dataflow avoids all transposes:
  phase 1: hT[128h, T] = sum_k Wi_tile[k128, h128].T @ xtT[k128, T]
  phase 2: y[128t, 512d] += hT[sl][:, t128].T @ Wo[sl*128, d512]

Everything is hardcoded for the fixed problem shape (B,S,D)=(4,2048,2048),
E=8 experts with hidden sizes 4096..11264, top-2 routing. If a routing
outcome ever exceeds the 1536-token block capacity, we fall back to a
pure-numpy path.
"""
import os
import sys

import numpy as np

D = 2048
E = 8
TOPK = 2
LN_EPS = 1e-5
HIDDENS = [int(4 * D * (0.5 + e / E)) for e in range(E)]
NCORES = 8

NSLOT = 15
SLOTW = [1536] * 9 + [1408] * 6
BLK = 1536                  # per-expert token block capacity
XCAP = 3072                 # two 1536 blocks per core
YTILES = XCAP // 128
# (expert_a, expert_b, units_a_per_core, units_b_per_core)
PAIRS = [(5, 2, 9, 6), (4, 3, 8, 7), (0, 7, 4, 11), (6, 1, 10, 5)]

_state: dict = {}


# ---------------------------------------------------------------- host math
def _routing(x, ln_g, ln_b, gate_W, gate_b, tpW1, tpb1, tpW2, tpb2):
    xf = x.reshape(-1, D).astype(np.float32)
    N = xf.shape[0]
    mu = xf.mean(-1, keepdims=True)
    xc = xf - mu
    var = np.mean(xc * xc, axis=-1, keepdims=True)
    xn = xc * (1.0 / np.sqrt(var + LN_EPS)) * ln_g + ln_b

    logits = xn @ gate_W + gate_b
    m = logits.max(-1, keepdims=True)
    e = np.exp(logits - m)
    probs = e / e.sum(-1, keepdims=True)

    tw = 1.0 / (1.0 + np.exp(-(np.maximum(xn @ tpW1 + tpb1, 0.0) @ tpW2 + tpb2)))
    eff_k = np.clip(np.round(tw.sum(-1)), 1, TOPK).astype(np.int32)

    top1 = probs.argmax(-1)
    pm = probs.copy()
    pm[np.arange(N), top1] = -np.inf
    top2 = pm.argmax(-1)
    p1 = probs[np.arange(N), top1]
    p2 = probs[np.arange(N), top2]

    m2 = (eff_k == 2).astype(np.float32)
    denom = p1 + m2 * p2 + np.float32(1e-8)
    w1 = p1 / denom
    w2 = (m2 * p2) / denom
    return xn, top1, top2, eff_k, w1, w2


def _gelu_np(v):
    try:
        from scipy.special import erf
    except ImportError:
        def erf(t):
            s = np.sign(t)
            u = 1.0 / (1.0 + 0.3275911 * np.abs(t))
            poly = u * (0.254829592 + u * (-0.284496736 + u * (
                1.421413741 + u * (-1.453152027 + u * 1.061405429))))
            return s * (1.0 - poly * np.exp(-t * t))
    return 0.5 * v * (1.0 + erf(v * np.float32(0.7071067811865476)))


def _numpy_fallback(x, xn, idxs, ws, Wi, bi, Wo, bo):
    out = np.zeros_like(xn)
    for e in range(E):
        idx, w = idxs[e], ws[e]
        if idx.size == 0:
            continue
        He = HIDDENS[e]
        h = _gelu_np(xn[idx] @ Wi[e][:, :He] + bi[e][:He])
        y = h @ Wo[e][:He, :] + bo[e]
        out[idx] += w[:, None] * y
    return (x + out.reshape(x.shape)).astype(np.float32)


def _to_bf16(a):
    import ml_dtypes
    return np.asarray(a).astype(ml_dtypes.bfloat16)


# ---------------------------------------------------------------- device
def _build_nc():
    import concourse.bass as bass
    import concourse.mybir as mybir
    from concourse import bacc, tile
    from contextlib import ExitStack

    bf16 = mybir.dt.bfloat16
    f32 = mybir.dt.float32
    i32 = mybir.dt.int32
    KT = D // 128
    POOL = mybir.EngineType.Pool
    DVE = mybir.EngineType.DVE

    nc = bacc.Bacc("TRN2", target_bir_lowering=False, debug=False,
                   num_devices=NCORES)
    xtT_d = nc.declare_dram_parameter("xtT", [D, XCAP], bf16, isOutput=False)
    wi_d = nc.declare_dram_parameter("wi", [NSLOT * 4, 128, D], bf16,
                                     isOutput=False)
    wo_d = nc.declare_dram_parameter("wo", [NSLOT * 4 * 128, D], bf16,
                                     isOutput=False)
    bi_d = nc.declare_dram_parameter("bi", [128, NSLOT * 4], f32,
                                     isOutput=False)
    # per-slot tables: [0,:] = xtT column offset, [1,:] = y_acc elem offset
    tbl_d = nc.declare_dram_parameter("tbl", [2, NSLOT], i32, isOutput=False)
    y_d = nc.declare_dram_parameter("y", [XCAP, D], bf16, isOutput=True)

    with tile.TileContext(nc) as tc, ExitStack() as ctx:
        const_p = ctx.enter_context(tc.tile_pool(name="const", bufs=1))
        xk_p = ctx.enter_context(tc.tile_pool(name="xk", bufs=16))
        wi_p = ctx.enter_context(tc.tile_pool(name="wi", bufs=6))
        wo_p = ctx.enter_context(tc.tile_pool(name="wo", bufs=6))
        ht_p = ctx.enter_context(tc.tile_pool(name="ht", bufs=8))
        acc_p = ctx.enter_context(tc.tile_pool(name="acc", bufs=1))
        ps_p = ctx.enter_context(tc.tile_pool(name="psum", bufs=8, space="PSUM"))

        bias_sb = const_p.tile([128, NSLOT * 4], f32)
        nc.sync.dma_start(out=bias_sb[:, :], in_=bi_d[:, :])
        tbl_sb = const_p.tile([2, NSLOT], i32)
        nc.sync.dma_start(out=tbl_sb[:, :], in_=tbl_d[:, :])

        y_acc = acc_p.tile([128, YTILES * D], bf16)
        nc.vector.memset(y_acc[:, :], 0.0)

        # job-table values into registers (POOL for DMAs, DVE for adds)
        _, xoffs = nc.values_load_multi_w_load_instructions(
            tbl_sb[0:1, :], engines=[POOL],
            min_val=0, max_val=BLK, skip_runtime_bounds_check=True)
        _, yoffs = nc.values_load_multi_w_load_instructions(
            tbl_sb[1:2, :], engines=[DVE],
            min_val=0, max_val=(BLK // 128) * D, skip_runtime_bounds_check=True)

        for j in range(NSLOT):
            W = SLOTW[j]
            xo, yo = xoffs[j], yoffs[j]
            wi_t = [wi_p.tile([128, D], bf16, tag="wi", name=f"wi_{j}_{sl}")
                    for sl in range(4)]
            wo_t = [wo_p.tile([128, D], bf16, tag="wo", name=f"wo_{j}_{sl}")
                    for sl in range(4)]
            for sl in range(4):
                nc.sync.dma_start(out=wi_t[sl][:, :], in_=wi_d[j * 4 + sl, :, :])
                r0 = (j * 4 + sl) * 128
                nc.sync.dma_start(out=wo_t[sl][:, :], in_=wo_d[r0:r0 + 128, :])
            ht = [ht_p.tile([128, W], bf16, tag="ht", name=f"ht_{j}_{sl}")
                  for sl in range(4)]
            # phase 1 over n-chunks of <=512 tokens
            nleft = W
            for n in range((W + 511) // 512):
                nW = min(512, nleft)
                nleft -= nW
                xk = []
                for k in range(KT):
                    t = xk_p.tile([128, 512], bf16, tag="xk",
                                  name=f"xk_{j}_{n}_{k}")
                    nc.gpsimd.dma_start(
                        out=t[:, :nW],
                        in_=xtT_d[k * 128:(k + 1) * 128,
                                  bass.ds(xo + n * 512, nW)])
                    xk.append(t)
                for sl in range(4):
                    ps = ps_p.tile([128, 512], f32, tag="ps",
                                   name=f"ps_{j}_{n}_{sl}")
                    for k in range(KT):
                        nc.tensor.matmul(
                            ps[:, :nW],
                            lhsT=wi_t[sl][:, k * 128:(k + 1) * 128],
                            rhs=xk[k][:, :nW],
                            start=(k == 0), stop=(k == KT - 1))
                    nc.scalar.activation(
                        ht[sl][:, n * 512:n * 512 + nW], ps[:, :nW],
                        mybir.ActivationFunctionType.Gelu,
                        bias=bias_sb[:, j * 4 + sl:j * 4 + sl + 1])
            # phase 2: y[tok, :] += sum_sl ht[sl].T @ wo[sl]
            for t in range(W // 128):
                for d4 in range(4):
                    py = ps_p.tile([128, 512], f32, tag="ps",
                                   name=f"py_{j}_{t}_{d4}")
                    for sl in range(4):
                        nc.tensor.matmul(
                            py[:, :],
                            lhsT=ht[sl][:, t * 128:(t + 1) * 128],
                            rhs=wo_t[sl][:, d4 * 512:(d4 + 1) * 512],
                            start=(sl == 0), stop=(sl == 3))
                    dst = y_acc[:, bass.ds(yo + t * D + d4 * 512, 512)]
                    nc.vector.tensor_add(dst, dst, py[:, :])
        for g in range(YTILES):
            nc.sync.dma_start(out=y_d[g * 128:(g + 1) * 128, :],
                              in_=y_acc[:, g * D:(g + 1) * D])
    nc.compile()
    return nc


def _get_nc():
    if "nc" not in _state:
        for p in ("/opt/trn_rl_repo", "/root/.axon_site/_ro/trn_rl_repo"):
            if os.path.isdir(p) and p not in sys.path:
                sys.path.append(p)
        _state["nc"] = _build_nc()
    return _state["nc"]


# ------------------------------------------------------------- host packing
def _core_units(core):
    """-> (expert_a, expert_b, [(expert, unit_idx)] for the 15 slots)."""
    a, b, na, nb = PAIRS[core // 2]
    h = core % 2
    slots = [(a, h * na + u) for u in range(na)] + \
            [(b, h * nb + u) for u in range(nb)]
    return a, b, slots


def _prep_weights(Wi, bi, Wo):
    """Per-core packed weight arrays in the layouts the kernel DMAs."""
    # per-expert tile packs: wt_e[s, p, k*128+c] = Wi[e][k*128+p, s*128+c]
    packs = []
    for e in range(E):
        w = Wi[e].astype(np.float32).reshape(KTN := D // 128, 128, -1, 128)
        wt = np.ascontiguousarray(w.transpose(2, 1, 0, 3)).reshape(-1, 128, D)
        packs.append(_to_bf16(wt))  # [HMAX/128, 128, D]
    wo16 = [_to_bf16(Wo[e]) for e in range(E)]
    bi32 = [np.asarray(bi[e], np.float32) for e in range(E)]

    wi_all, wo_all, bi_all, tbl_all = [], [], [], []
    for c in range(NCORES):
        a, b, slots = _core_units(c)
        wi_rows, wo_rows, bi_cols, xoff, yoff = [], [], [], [], []
        for (e, u) in slots:
            s0 = u * 4
            wi_rows.append(packs[e][s0:s0 + 4])          # [4,128,D]
            wo_rows.append(wo16[e][s0 * 128:(s0 + 4) * 128])  # [512,D]
            bi_cols.append(bi32[e][s0 * 128:(s0 + 4) * 128].reshape(4, 128).T)
            base = 0 if e == a else BLK
            xoff.append(base)
            yoff.append((base // 128) * D)
        wi_all.append(np.concatenate(wi_rows, axis=0))
        wo_all.append(np.concatenate(wo_rows, axis=0))
        bi_all.append(np.concatenate(bi_cols, axis=1))   # [128, 60]
        tbl_all.append(np.array([xoff, yoff], np.int32))
    return wi_all, wo_all, bi_all, tbl_all


def _run_device(xtT_all, wi_all, wo_all, bi_all, tbl_all):
    from concourse.bass_utils import run_bass_kernel_spmd

    nc = _get_nc()
    in_maps = [
        {"xtT": xtT_all[c], "wi": wi_all[c], "wo": wo_all[c],
         "bi": bi_all[c], "tbl": tbl_all[c]}
        for c in range(NCORES)
    ]
    res = run_bass_kernel_spmd(nc, in_maps, list(range(NCORES)))
    _state["exec_time_ns"] = getattr(res, "exec_time_ns", None)
    return [r["y"] for r in res.results]


# ---------------------------------------------------------------- entry point
def kernel(x, ln_g, ln_b, gate_W, gate_b, tpW1, tpb1, tpW2, tpb2,
           Wi, bi, Wo, bo):
    x = np.asarray(x, np.float32)
    xn, top1, top2, eff_k, w1, w2 = _routing(
        x, ln_g, ln_b, gate_W, gate_b, tpW1, tpb1, tpW2, tpb2)

    idxs, ws = [], []
    for e in range(E):
        s1 = np.nonzero(top1 == e)[0]
        s2 = np.nonzero((top2 == e) & (eff_k == 2))[0]
        idx = np.concatenate([s1, s2])
        w = np.concatenate([w1[s1], w2[s2]]).astype(np.float32)
        idxs.append(idx)
        ws.append(w)

    if max(i.size for i in idxs) > BLK:
        return _numpy_fallback(x, xn, idxs, ws, Wi, bi, Wo, bo)

    try:
        if "wi_all" not in _state or _state.get("w_id") != id(Wi):
            (_state["wi_all"], _state["wo_all"], _state["bi_all"],
             _state["tbl_all"]) = _prep_weights(Wi, bi, Wo)
            _state["w_id"] = id(Wi)
        xnT = np.ascontiguousarray(xn.T)  # [D, N] f32
        xtT_all = []
        for c in range(NCORES):
            a, b, _ = _core_units(c)
            blk = np.zeros((D, XCAP), np.float32)
            blk[:, :idxs[a].size] = xnT[:, idxs[a]]
            blk[:, BLK:BLK + idxs[b].size] = xnT[:, idxs[b]]
            xtT_all.append(_to_bf16(blk))
        y_all = _run_device(xtT_all, _state["wi_all"], _state["wo_all"],
                            _state["bi_all"], _state["tbl_all"])
    except Exception:
        import traceback
        traceback.print_exc()
        return _numpy_fallback(x, xn, idxs, ws, Wi, bi, Wo, bo)

    # merge pair partials, apply gate weights + bo, scatter + residual
    out = np.zeros_like(xn)
    for p, (a, b, _, _) in enumerate(PAIRS):
        cA, cB = 2 * p, 2 * p + 1
        for e, base in ((a, 0), (b, BLK)):
            idx, w = idxs[e], ws[e]
            if idx.size == 0:
                continue
            y = (np.asarray(y_all[cA][base:base + idx.size], np.float32) +
                 np.asarray(y_all[cB][base:base + idx.size], np.float32) +
                 bo[e])
            out[idx] += w[:, None] * y
    return (x + out.reshape(x.shape)).astype(np.float32)
